# revision 12
# baseline (speedup 1.0000x reference)
"""DeepRNN (4-layer ReLU RNN, B=32 T=1024 H=512) on 8 trn2 NeuronCores.

Shipped strategy ("pl"): 4-stage layer pipeline x 2 batch replicas. Each core
owns one layer; chunks of 32 timesteps flow down the pipeline via masked pair
AllGathers (contiguous pairs + stride-4 pairs, the only group shapes the
runtime accepts), 4-deep slot parity so a recv never waits on an in-flight
collective. Per chunk, the input projection (bias folded in through a 5th
contraction block) is accumulated directly into PSUM and the recurrence
matmuls accumulate on top; the per-step relu evac is a single tensor_scalar
per jt-half with separate psum tiles per half so evac overlaps the other
half's matmuls. fc runs as a tail phase. Numerics: bf16 operands, fp32 PSUM
accumulate; rel err ~9e-3 vs fp32 reference.

Older variants kept for reference: "dp" (data-parallel, each core runs all 4
layers on B/8 lanes) and "cc" (earlier pipeline attempt; its group shapes
hang the runtime - do not use).
"""
import numpy as np
import ml_dtypes

import concourse.bass as bass
import concourse.bacc as bacc
import concourse.mybir as mybir
import concourse.tile as tile
from concourse.bass_utils import run_bass_kernel_spmd

# problem dims (hardcoded per contract)
B, T, H, O, L = 32, 1024, 512, 512, 4
P = 128
KT = JT = IT = OT = H // P          # 4 tiles per 512 dim
NCORES = 8
NREP = 2                            # batch replicas
BL = B // NREP                      # 16 batch lanes per core
CH = 32                             # timesteps per chunk
NCHUNK = T // CH                    # 32
DELAY = 2                           # iterations between produce and consume
NITER = NCHUNK + DELAY * (L - 1)    # 38
CB = CH * BL                        # 512 cols per chunk

BF = mybir.dt.bfloat16
F32 = mybir.dt.float32

GROUPS_A = [[0, 1], [2, 3], [4, 5], [6, 7]]
GROUPS_B = [[0, 7], [1, 2], [3, 4], [5, 6]]

# relu/gating DVE forms: "ts" uses tensor_scalar (1 sync-wait slot),
# "tt" uses tensor_tensor against a zero tile (2 wait slots).
USE_TT_FOR_MAX = True


def _build():
    nc = bacc.Bacc("TRN2", target_bir_lowering=False, debug=False, num_devices=8)
    xin = nc.declare_dram_parameter("xin", [NCHUNK, P, IT, CB], BF, isOutput=False)
    wiT = nc.declare_dram_parameter("wiT", [P, IT * JT * P], BF, isOutput=False)
    whT = nc.declare_dram_parameter("whT", [P, KT * JT * P], BF, isOutput=False)
    wfcT = nc.declare_dram_parameter("wfcT", [P, KT * OT * P], BF, isOutput=False)
    bias = nc.declare_dram_parameter("bias", [P, JT], F32, isOutput=False)
    bfc = nc.declare_dram_parameter("bfc", [P, OT], F32, isOutput=False)
    valid = nc.declare_dram_parameter("valid", [P, NITER], F32, isOutput=False)
    kappa = nc.declare_dram_parameter("kappa", [P, NITER], F32, isOutput=False)
    smask = nc.declare_dram_parameter("smask", [P, 2], F32, isOutput=False)
    out = nc.declare_dram_parameter("out", [NCHUNK + 1, P, OT, CB], F32, isOutput=True)

    # separate tensors per parity slot so Tile's name-level DRAM dependency
    # tracking doesn't serialize iteration k's consume behind iteration
    # k-1's collective (they touch different parities)
    cc_in = [nc.dram_tensor(f"cc_in{s}", [2, P, JT * CB], BF) for s in range(2)]
    cc_outA = [nc.dram_tensor(f"cc_outA{s}", [2, P, JT * CB], BF) for s in range(2)]
    cc_outB = [nc.dram_tensor(f"cc_outB{s}", [2, P, JT * CB], BF) for s in range(2)]

    with tile.TileContext(nc) as tc:
        with (
            tc.tile_pool(name="const", bufs=1) as cpool,
            tc.tile_pool(name="state", bufs=1) as spool,
            tc.tile_pool(name="io2", bufs=2) as io2,
            tc.tile_pool(name="hsq", bufs=2) as hsq,
            tc.tile_pool(name="step", bufs=2) as stp,
            tc.tile_pool(name="psP", bufs=2, space="PSUM") as psP,
            tc.tile_pool(name="psR", bufs=2, space="PSUM") as psR,
            tc.tile_pool(name="psF", bufs=2, space="PSUM") as psF,
        ):
            # ---- constants to SBUF, with wait-absorbing dummy reads ----
            wiT_sb = cpool.tile([P, IT * JT * P], BF)
            whT_sb = cpool.tile([P, KT * JT * P], BF)
            wfcT_sb = cpool.tile([P, KT * OT * P], BF)
            bias_sb = cpool.tile([P, JT], F32)
            bfc_sb = cpool.tile([P, OT], F32)
            valid_sb = cpool.tile([P, NITER], F32)
            kappa_sb = cpool.tile([P, NITER], F32)
            smask_sb = cpool.tile([P, 2], F32)
            nc.sync.dma_start(wiT_sb[:], wiT[:])
            nc.sync.dma_start(whT_sb[:], whT[:])
            nc.sync.dma_start(wfcT_sb[:], wfcT[:])
            nc.sync.dma_start(bias_sb[:], bias[:])
            nc.sync.dma_start(bfc_sb[:], bfc[:])
            nc.sync.dma_start(valid_sb[:], valid[:])
            nc.sync.dma_start(kappa_sb[:], kappa[:])
            nc.sync.dma_start(smask_sb[:], smask[:])
            dump = spool.tile([P, 8], F32)
            for i, t_ in enumerate(
                [wiT_sb, whT_sb, wfcT_sb, bias_sb, bfc_sb, valid_sb, kappa_sb,
                 smask_sb]):
                nc.vector.tensor_copy(dump[:, i:i + 1], t_[:, 0:1])

            zero_sb = cpool.tile([P, JT * BL], F32)
            nc.vector.memset(zero_sb[:], 0.0)
            zrow = cpool.tile([P, JT * CB], BF)
            nc.vector.memset(zrow[:], 0.0)
            # zero the collective landing zones (consumed at iters 0/1
            # before any collective has produced data)
            for slot in range(2):
                for g in range(2):
                    nc.sync.dma_start(cc_outA[slot][g], zrow[:])
                    nc.sync.dma_start(cc_outB[slot][g], zrow[:])

            # persistent recurrent state
            hcur = spool.tile([P, KT, BL], BF)
            nc.vector.memset(hcur[:], 0.0)

            for k in range(NITER):
                slot = k % 2
                # ---- assemble input chunk: x + recvA + recvB ----
                xb = io2.tile([P, IT, CB], BF, tag="xb")
                nc.sync.dma_start(xb[:], xin[min(k, NCHUNK - 1)])
                ra = io2.tile([P, IT * CB], BF, tag="ra")
                nc.sync.dma_start(ra[:], cc_outA[slot][0])
                rb = io2.tile([P, IT * CB], BF, tag="rb")
                nc.sync.dma_start(rb[:], cc_outB[slot][0])
                t1 = io2.tile([P, IT * CB], BF, tag="t1")
                nc.vector.tensor_tensor(
                    t1[:], xb[:].rearrange("p i c -> p (i c)"), ra[:],
                    mybir.AluOpType.add)
                insb = io2.tile([P, IT, CB], BF, tag="insb")
                nc.vector.tensor_tensor(
                    insb[:].rearrange("p i c -> p (i c)"), t1[:], rb[:],
                    mybir.AluOpType.add)

                # ---- input projection (gated by valid[k], bias folded) ----
                xw = io2.tile([P, JT, CH, BL], F32, tag="xw")
                for jt in range(JT):
                    pp = psP.tile([P, CB], F32, tag="pp")
                    for it in range(IT):
                        nc.tensor.matmul(
                            pp[:],
                            wiT_sb[:, (it * JT + jt) * P:(it * JT + jt + 1) * P],
                            insb[:, it, :],
                            start=(it == 0), stop=(it == IT - 1))
                    nc.vector.tensor_scalar(
                        xw[:, jt, :, :].rearrange("p c b -> p (c b)"), pp[:],
                        bias_sb[:, jt:jt + 1], valid_sb[:, k:k + 1],
                        mybir.AluOpType.add, mybir.AluOpType.mult)

                # ---- recurrence over the chunk ----
                hseq = hsq.tile([P, JT, CH, BL], BF, tag="hseq")
                nc.vector.tensor_scalar_mul(
                    hcur[:].rearrange("p k b -> p (k b)"),
                    hcur[:].rearrange("p k b -> p (k b)"),
                    kappa_sb[:, k:k + 1])
                for c in range(CH):
                    pt = psR.tile([P, JT * BL], F32, tag="pt")
                    for jt in range(JT):
                        for kt in range(KT):
                            rhs = (hcur[:, kt, :] if c == 0
                                   else hseq[:, kt, c - 1, :])
                            nc.tensor.matmul(
                                pt[:, jt * BL:(jt + 1) * BL],
                                whT_sb[:, (kt * JT + jt) * P:(kt * JT + jt + 1) * P],
                                rhs,
                                start=(kt == 0), stop=(kt == KT - 1))
                    tmp = stp.tile([P, JT, BL], F32, tag="tmp")
                    nc.vector.tensor_tensor(
                        tmp[:], pt[:].rearrange("p (j b) -> p j b", j=JT),
                        xw[:, :, c, :], mybir.AluOpType.add)
                    dst = hseq[:, :, c, :]
                    if USE_TT_FOR_MAX:
                        nc.vector.tensor_tensor(
                            dst, tmp[:],
                            zero_sb[:].rearrange("p (j b) -> p j b", j=JT),
                            mybir.AluOpType.max)
                    else:
                        nc.vector.tensor_scalar_max(dst, tmp[:], 0.0)
                nc.vector.tensor_copy(hcur[:], hseq[:, :, CH - 1, :])

                # ---- fc head (only stage-3 cores' results are kept) ----
                osb = io2.tile([P, OT, CB], F32, tag="osb")
                for ot in range(OT):
                    fp = psF.tile([P, CB], F32, tag="fp")
                    for ht in range(KT):
                        nc.tensor.matmul(
                            fp[:],
                            wfcT_sb[:, (ht * OT + ot) * P:(ht * OT + ot + 1) * P],
                            hseq[:, ht, :, :].rearrange("p c b -> p (c b)"),
                            start=(ht == 0), stop=(ht == KT - 1))
                    nc.vector.tensor_scalar_add(
                        osb[:, ot, :], fp[:], bfc_sb[:, ot:ot + 1])
                oslice = k - DELAY * (L - 1)
                if not (0 <= oslice < NCHUNK):
                    oslice = NCHUNK
                nc.sync.dma_start(out[oslice], osb[:])

                # ---- send my chunk to successor (masked 2-slot write) ----
                snd = io2.tile([P, 2, JT * CB], BF, tag="snd")
                hflat = hseq[:].rearrange("p j c b -> p (j c b)")
                nc.vector.tensor_scalar_mul(snd[:, 0, :], hflat, smask_sb[:, 0:1])
                nc.vector.tensor_scalar_mul(snd[:, 1, :], hflat, smask_sb[:, 1:2])
                nc.sync.dma_start(cc_in[slot][0], snd[:, 0, :])
                nc.sync.dma_start(cc_in[slot][1], snd[:, 1, :])
                nc.gpsimd.collective_compute(
                    "AllGather", mybir.AluOpType.bypass, GROUPS_A,
                    ins=[cc_in[slot][0]], outs=[cc_outA[slot][:]])
                nc.gpsimd.collective_compute(
                    "AllGather", mybir.AluOpType.bypass, GROUPS_B,
                    ins=[cc_in[slot][1]], outs=[cc_outB[slot][:]])
    nc.compile()
    return nc


BL8 = B // NCORES                    # 4 batch lanes per core in DP mode
TB = T * BL8                         # 4096 cols


def _build_dp():
    """Data-parallel fallback: each core runs all 4 layers + fc on B/8=4
    batch lanes. No collectives."""
    nc = bacc.Bacc("TRN2", target_bir_lowering=False, debug=False,
                   num_devices=8)
    xin = nc.declare_dram_parameter("xin", [P, IT, TB], BF, isOutput=False)
    wiT = nc.declare_dram_parameter("wiT", [P, L * IT * JT * P], BF, isOutput=False)
    whT = nc.declare_dram_parameter("whT", [P, L * KT * JT * P], BF, isOutput=False)
    wfcT = nc.declare_dram_parameter("wfcT", [P, KT * OT * P], BF, isOutput=False)
    bias = nc.declare_dram_parameter("bias", [P, L * JT], F32, isOutput=False)
    bfc = nc.declare_dram_parameter("bfc", [P, OT], F32, isOutput=False)
    out = nc.declare_dram_parameter("out", [P, OT, TB], F32, isOutput=True)

    NCOL = 512                       # psum bank limit: 512 fp32 out cols
    NSPL = TB // NCOL                # 4

    with tile.TileContext(nc) as tc:
        with (
            tc.tile_pool(name="const", bufs=1) as cpool,
            tc.tile_pool(name="act", bufs=1) as apool,
            tc.tile_pool(name="step", bufs=2) as stp,
            tc.tile_pool(name="psP", bufs=2, space="PSUM") as psP,
            tc.tile_pool(name="psR", bufs=2, space="PSUM") as psR,
        ):
            wiT_sb = cpool.tile([P, L * IT * JT * P], BF)
            whT_sb = cpool.tile([P, L * KT * JT * P], BF)
            wfcT_sb = cpool.tile([P, KT * OT * P], BF)
            bias_sb = cpool.tile([P, L * JT], F32)
            bfc_sb = cpool.tile([P, OT], F32)
            nc.sync.dma_start(wiT_sb[:], wiT[:])
            nc.sync.dma_start(whT_sb[:], whT[:])
            nc.sync.dma_start(wfcT_sb[:], wfcT[:])
            nc.sync.dma_start(bias_sb[:], bias[:])
            nc.sync.dma_start(bfc_sb[:], bfc[:])
            zero_sb = cpool.tile([P, JT, BL8], F32)
            nc.vector.memset(zero_sb[:], 0.0)
            zbf = cpool.tile([P, KT, BL8], BF)
            nc.vector.memset(zbf[:], 0.0)

            cur = apool.tile([P, IT, TB], BF, tag="cur")
            nc.sync.dma_start(cur[:], xin[:])

            for l in range(L):
                # input projection for the whole sequence
                xw = apool.tile([P, JT, T, BL8], BF, tag=f"xw{l % 2}")
                cur_f = (cur[:].rearrange("p i t -> p (i t)") if l == 0
                         else cur[:].rearrange("p j t b -> p (j t b)"))
                for jt in range(JT):
                    for sp in range(NSPL):
                        pp = psP.tile([P, NCOL], F32, tag="pp")
                        for it in range(IT):
                            nc.tensor.matmul(
                                pp[:],
                                wiT_sb[:, (l * 16 + it * JT + jt) * P:(l * 16 + it * JT + jt + 1) * P],
                                cur_f[:, it * TB + sp * NCOL:it * TB + (sp + 1) * NCOL],
                                start=(it == 0), stop=(it == IT - 1))
                        xw_f = xw[:].rearrange("p j t b -> p (j t b)")
                        nc.vector.tensor_scalar_add(
                            xw_f[:, jt * TB + sp * NCOL:jt * TB + (sp + 1) * NCOL],
                            pp[:], bias_sb[:, l * JT + jt:l * JT + jt + 1])
                # recurrence
                hseq = apool.tile([P, JT, T, BL8], BF, tag=f"hs{l % 2}")
                for t in range(T):
                    pt = psR.tile([P, JT * BL8], F32, tag="pt")
                    for jt in range(JT):
                        for kt in range(KT):
                            nc.tensor.matmul(
                                pt[:, jt * BL8:(jt + 1) * BL8],
                                whT_sb[:, (l * 16 + kt * JT + jt) * P:(l * 16 + kt * JT + jt + 1) * P],
                                hseq[:, kt, t - 1, :] if t > 0 else zbf[:, kt, :],
                                start=(kt == 0), stop=(kt == KT - 1))
                    tmp = stp.tile([P, JT, BL8], F32, tag="tmp")
                    nc.vector.tensor_tensor(
                        tmp[:], pt[:].rearrange("p (j b) -> p j b", j=JT),
                        xw[:, :, t, :], mybir.AluOpType.add)
                    nc.vector.tensor_tensor(
                        hseq[:, :, t, :], tmp[:], zero_sb[:],
                        mybir.AluOpType.max)
                cur = hseq

            # fc head
            for ot in range(OT):
                for sp in range(NSPL):
                    fp = psP.tile([P, NCOL], F32, tag="pp")
                    cur_f = cur[:].rearrange("p h t b -> p (h t b)")
                    for ht in range(KT):
                        nc.tensor.matmul(
                            fp[:],
                            wfcT_sb[:, (ht * OT + ot) * P:(ht * OT + ot + 1) * P],
                            cur_f[:, ht * TB + sp * NCOL:ht * TB + (sp + 1) * NCOL],
                            start=(ht == 0), stop=(ht == KT - 1))
                    osb = stp.tile([P, NCOL], F32, tag="osb")
                    nc.vector.tensor_scalar_add(
                        osb[:], fp[:], bfc_sb[:, ot:ot + 1])
                    nc.sync.dma_start(
                        out[:, ot, sp * NCOL:(sp + 1) * NCOL], osb[:])
    nc.compile()
    return nc


CH2 = 32                             # rec chunk (psum-resident xw)


def _build_dp2():
    """DP with xw kept in PSUM: proj writes psum, rec matmuls accumulate
    on top (start=False), relu+bias evac is one tensor_scalar per jt."""
    nc = bacc.Bacc("TRN2", target_bir_lowering=False, debug=False,
                   num_devices=8)
    xin = nc.declare_dram_parameter("xin", [P, IT, TB], BF, isOutput=False)
    wiT = nc.declare_dram_parameter("wiT", [P, L * IT * JT * P], BF, isOutput=False)
    whT = nc.declare_dram_parameter("whT", [P, L * KT * JT * P], BF, isOutput=False)
    wfcT = nc.declare_dram_parameter("wfcT", [P, KT * OT * P], BF, isOutput=False)
    bias = nc.declare_dram_parameter("bias", [P, L * JT], F32, isOutput=False)
    bfc = nc.declare_dram_parameter("bfc", [P, OT], F32, isOutput=False)
    out = nc.declare_dram_parameter("out", [P, OT, TB], F32, isOutput=True)

    NCOL = 512
    NSPL = TB // NCOL
    NCH = T // CH2

    with tile.TileContext(nc) as tc:
        with (
            tc.tile_pool(name="const", bufs=1) as cpool,
            tc.tile_pool(name="act", bufs=1) as apool,
            tc.tile_pool(name="step", bufs=2) as stp,
            tc.tile_pool(name="psP", bufs=2, space="PSUM") as psP,
            tc.tile_pool(name="psX", bufs=2, space="PSUM") as psX,
        ):
            wiT_sb = cpool.tile([P, L * IT * JT * P], BF)
            whT_sb = cpool.tile([P, L * KT * JT * P], BF)
            wfcT_sb = cpool.tile([P, KT * OT * P], BF)
            bias_sb = cpool.tile([P, L * JT], F32)
            bfc_sb = cpool.tile([P, OT], F32)
            nc.sync.dma_start(wiT_sb[:], wiT[:])
            nc.sync.dma_start(whT_sb[:], whT[:])
            nc.sync.dma_start(wfcT_sb[:], wfcT[:])
            nc.sync.dma_start(bias_sb[:], bias[:])
            nc.sync.dma_start(bfc_sb[:], bfc[:])
            zbf = cpool.tile([P, KT, BL8], BF)
            nc.vector.memset(zbf[:], 0.0)

            cur = apool.tile([P, IT, TB], BF, tag="cur")
            nc.sync.dma_start(cur[:], xin[:])

            for l in range(L):
                cur_f = (cur[:].rearrange("p i t -> p (i t)") if l == 0
                         else cur[:].rearrange("p j t b -> p (j t b)"))
                hseq = apool.tile([P, JT, T, BL8], BF, tag=f"hs{l % 2}")
                for kc in range(NCH):
                    # proj chunk directly into psum [P, (jt, c, b)]
                    xps = psX.tile([P, JT, CH2, BL8], F32, tag="xps")
                    cb0 = kc * CH2 * BL8
                    for jt in range(JT):
                        for it in range(IT):
                            nc.tensor.matmul(
                                xps[:, jt, :, :],
                                wiT_sb[:, (l * 16 + it * JT + jt) * P:
                                       (l * 16 + it * JT + jt + 1) * P],
                                cur_f[:, it * TB + cb0:it * TB + cb0 + CH2 * BL8],
                                start=(it == 0), stop=False,
                                skip_group_check=True)
                    # recurrence, accumulating onto the psum xw
                    for c in range(CH2):
                        t = kc * CH2 + c
                        for jt in range(JT):
                            for kt in range(KT):
                                rhs = (hseq[:, kt, t - 1, :] if t > 0
                                       else zbf[:, kt, :])
                                nc.tensor.matmul(
                                    xps[:, jt, c, :],
                                    whT_sb[:, (l * 16 + kt * JT + jt) * P:
                                           (l * 16 + kt * JT + jt + 1) * P],
                                    rhs,
                                    start=False, stop=(kt == KT - 1),
                                    skip_group_check=True)
                            nc.vector.tensor_scalar(
                                hseq[:, jt, t, :], xps[:, jt, c, :],
                                bias_sb[:, l * JT + jt:l * JT + jt + 1], 0.0,
                                mybir.AluOpType.add, mybir.AluOpType.max)
                cur = hseq

            for ot in range(OT):
                cur_f = cur[:].rearrange("p h t b -> p (h t b)")
                for sp in range(NSPL):
                    fp = psP.tile([P, NCOL], F32, tag="pp")
                    for ht in range(KT):
                        nc.tensor.matmul(
                            fp[:],
                            wfcT_sb[:, (ht * OT + ot) * P:(ht * OT + ot + 1) * P],
                            cur_f[:, ht * TB + sp * NCOL:ht * TB + (sp + 1) * NCOL],
                            start=(ht == 0), stop=(ht == KT - 1))
                    osb = stp.tile([P, NCOL], F32, tag="osb")
                    nc.vector.tensor_scalar_add(
                        osb[:], fp[:], bfc_sb[:, ot:ot + 1])
                    nc.sync.dma_start(
                        out[:, ot, sp * NCOL:(sp + 1) * NCOL], osb[:])
    nc.compile()
    return nc


def _kernel_dp(x, W_ih, W_hh, b_ih, b_hh, W_fc, b_fc):
    global _NC_CACHE_DP
    in_maps = []
    wiT = np.concatenate([_pack_w(W_ih[l]) for l in range(L)], axis=1).astype(ml_dtypes.bfloat16)
    whT = np.concatenate([_pack_w(W_hh[l]) for l in range(L)], axis=1).astype(ml_dtypes.bfloat16)
    wfcT = _pack_w(W_fc).astype(ml_dtypes.bfloat16)
    bias = np.ascontiguousarray(
        (b_ih + b_hh).reshape(L * JT, P).T).astype(np.float32)
    bfc_a = np.ascontiguousarray(b_fc.reshape(OT, P).T).astype(np.float32)
    for core in range(NCORES):
        b0 = core * BL8
        xh = x[b0:b0 + BL8]                           # [BL8, T, H]
        xr = xh.reshape(BL8, T, IT, P).transpose(3, 2, 1, 0)   # p it t b
        xin = np.ascontiguousarray(
            xr.reshape(P, IT, TB)).astype(ml_dtypes.bfloat16)
        in_maps.append({"xin": xin, "wiT": wiT, "whT": whT, "wfcT": wfcT,
                        "bias": bias, "bfc": bfc_a})
    if _NC_CACHE_DP is None:
        _NC_CACHE_DP = _build_dp()
    global _last_in_maps, _last_nc
    _last_in_maps, _last_nc = in_maps, _NC_CACHE_DP
    res = run_bass_kernel_spmd(_NC_CACHE_DP, in_maps,
                               core_ids=list(range(NCORES)))
    y = np.empty((B, T, O), np.float32)
    for core in range(NCORES):
        arr = res.results[core]["out"]                # [P, OT, TB]
        arr = arr.reshape(P, OT, T, BL8).transpose(3, 2, 1, 0)
        y[core * BL8:(core + 1) * BL8] = arr.reshape(BL8, T, O)
    return y


_NC_CACHE_DP = None


def _pack_w(w):
    """[512(out j), 512(in k)] -> [P, (kt*JT+jt)*P + j] = w[jt*P+j, kt*P+p]"""
    r = w.reshape(JT, P, KT, P)            # [jt, j, kt, p]
    r = r.transpose(3, 2, 0, 1)            # [p, kt, jt, j]
    return np.ascontiguousarray(r.reshape(P, KT * JT * P))


def _bcast(v):
    return np.ascontiguousarray(np.broadcast_to(v, (P,) + v.shape).astype(np.float32))


_NC_CACHE = None


def _kernel_cc(x, W_ih, W_hh, b_ih, b_hh, W_fc, b_fc):
    global _NC_CACHE
    x = np.asarray(x); W_ih = np.asarray(W_ih); W_hh = np.asarray(W_hh)
    b_ih = np.asarray(b_ih); b_hh = np.asarray(b_hh)
    W_fc = np.asarray(W_fc); b_fc = np.asarray(b_fc)

    in_maps = []
    for core in range(NCORES):
        s = core % 4          # pipeline stage == layer index
        rep = core // 4       # batch replica
        b0 = rep * BL

        if s == 0:
            xh = x[b0:b0 + BL]                                   # [BL, T, H]
            xr = xh.reshape(BL, NCHUNK, CH, IT, P)               # b k c i p
            xr = xr.transpose(1, 4, 3, 2, 0)                     # k p i c b
            xin = np.ascontiguousarray(
                xr.reshape(NCHUNK, P, IT, CB)).astype(ml_dtypes.bfloat16)
        else:
            xin = np.zeros((NCHUNK, P, IT, CB), ml_dtypes.bfloat16)

        wiT = _pack_w(W_ih[s]).astype(ml_dtypes.bfloat16)
        whT = _pack_w(W_hh[s]).astype(ml_dtypes.bfloat16)
        wfcT = _pack_w(W_fc).astype(ml_dtypes.bfloat16)
        # bias[j] with j = jt*P + p  ->  [P, JT] element [p, jt]
        bias = np.ascontiguousarray(
            (b_ih[s] + b_hh[s]).reshape(JT, P).T).astype(np.float32)
        bfc_a = np.ascontiguousarray(b_fc.reshape(OT, P).T).astype(np.float32)

        val = np.zeros(NITER, np.float32)
        kap = np.zeros(NITER, np.float32)
        lo = DELAY * s
        val[lo:lo + NCHUNK] = 1.0
        kap[lo + 1:lo + NCHUNK] = 1.0
        succ = (core + 1) % NCORES
        sA = 1.0 if (core % 2 == 0 and succ % 4 != 0) else 0.0
        sB = 1.0 if (core % 2 == 1 and succ % 4 != 0) else 0.0

        in_maps.append({
            "xin": xin,
            "wiT": wiT, "whT": whT, "wfcT": wfcT,
            "bias": bias,
            "bfc": bfc_a,
            "valid": _bcast(val),
            "kappa": _bcast(kap),
            "smask": _bcast(np.array([sA, sB], np.float32)),
        })

    global _last_in_maps
    _last_in_maps = in_maps
    if _NC_CACHE is None:
        _NC_CACHE = _build()
    res = run_bass_kernel_spmd(_NC_CACHE, in_maps, core_ids=list(range(NCORES)))

    y = np.empty((B, T, O), np.float32)
    for rep in range(NREP):
        arr = res.results[3 + 4 * rep]["out"][:NCHUNK]       # [k, P, OT, CB]
        arr = arr.reshape(NCHUNK, P, OT, CH, BL)
        arr = arr.transpose(4, 0, 3, 2, 1)                   # b k c ot p
        y[rep * BL:(rep + 1) * BL] = arr.reshape(BL, T, O)
    return y


def _bcast_rows(a):
    """per-partition table already [P, ...] -> passthrough; 1d -> broadcast"""
    a = np.asarray(a, np.float32)
    if a.ndim == 1:
        return _bcast(a)
    return np.ascontiguousarray(a)


# ---------------------------------------------------------------------------
# pl: 4-stage layer pipeline x 2 replicas with pair-collective handoff.
#
# Chains: 0 -A-> 1 -S-> 5 -A-> 4   and   2 -A-> 3 -S-> 7 -A-> 6
# (A = contiguous pairs [[0,1],[2,3],[4,5],[6,7]],
#  S = stride-4 pairs  [[0,4],[1,5],[2,6],[3,7]] — the only group shapes the
#  runtime accepts; sends and receives are masked per-core via dram data so
#  the SPMD instruction stream stays uniform.)
#
# Each iteration k: rec(chunk k) on psum buffer k%2 (xw pre-accumulated by
# proj during iter k-1, bias folded in via a 5th contraction block), proj for
# chunk k+1 interleaved into the late rec steps, hseq chunk sent to the
# successor stage via 2 pair AllGathers with 3-deep slot parity (DELAY=3).
# fc runs as a tail phase over hbuf after the pipeline drains.
# ---------------------------------------------------------------------------
PCH = 32                                # pl: timesteps per chunk
PNCHUNK = T // PCH                      # 32
PDELAY = 4
PNITER = PNCHUNK + PDELAY * (L - 1)     # 41
PCB = PCH * BL                          # 512 cols per chunk

PL_STAGE = {0: 0, 1: 1, 5: 2, 4: 3, 2: 0, 3: 1, 7: 2, 6: 3}
PL_REP = {0: 0, 1: 0, 5: 0, 4: 0, 2: 1, 3: 1, 7: 1, 6: 1}
PL_GROUPS_A = [[0, 1], [2, 3], [4, 5], [6, 7]]
PL_GROUPS_S = [[0, 4], [1, 5], [2, 6], [3, 7]]
PL_SA = (0, 2, 5, 7)                    # A-collective senders
PL_SS = (1, 3)                          # S-collective senders
PL_MA0 = (1, 3)                         # read A member 0 (pred is lower pair member)
PL_MA1 = (4, 6)                         # read A member 1
PL_MS0 = (5, 7)                         # read S member 0
PL_OUTCORE = {0: 4, 1: 6}               # replica -> stage-3 core


def _build_pl():
    nc = bacc.Bacc("TRN2", target_bir_lowering=False, debug=False,
                   num_devices=8)
    xin = nc.declare_dram_parameter("xin", [PNCHUNK, P, IT, PCB], BF, isOutput=False)
    wiT = nc.declare_dram_parameter("wiT", [P, (IT + 1) * JT * P], BF, isOutput=False)
    whT = nc.declare_dram_parameter("whT", [P, KT * JT * P], BF, isOutput=False)
    wfcT = nc.declare_dram_parameter("wfcT", [P, (KT + 1) * OT * P], BF, isOutput=False)
    kappa = nc.declare_dram_parameter("kappa", [P, PNITER], F32, isOutput=False)
    smask = nc.declare_dram_parameter("smask", [P, 2], F32, isOutput=False)
    rmask = nc.declare_dram_parameter("rmask", [P, 3], F32, isOutput=False)
    out = nc.declare_dram_parameter("out", [PNCHUNK, P, OT, PCB], F32, isOutput=True)

    hbuf = nc.dram_tensor("hbuf", [PNCHUNK, P, JT * PCB], BF)
    cc_inA = [nc.dram_tensor(f"cc_inA{s}", [P, JT * PCB], BF) for s in range(4)]
    cc_inS = [nc.dram_tensor(f"cc_inS{s}", [P, JT * PCB], BF) for s in range(4)]
    cc_outA = [nc.dram_tensor(f"cc_outA{s}", [2, P, JT * PCB], BF) for s in range(4)]
    cc_outS = [nc.dram_tensor(f"cc_outS{s}", [2, P, JT * PCB], BF) for s in range(4)]

    with tile.TileContext(nc) as tc:
        with (
            tc.tile_pool(name="const", bufs=1) as cpool,
            tc.tile_pool(name="io2", bufs=2) as io2,
            tc.tile_pool(name="hsq", bufs=2) as hsq,
        ):
            # constants
            wiT_sb = cpool.tile([P, (IT + 1) * JT * P], BF)
            whT_sb = cpool.tile([P, KT * JT * P], BF)
            wfcT_sb = cpool.tile([P, (KT + 1) * OT * P], BF)
            kappa_sb = cpool.tile([P, PNITER], F32)
            smask_sb = cpool.tile([P, 2], F32)
            rmask_sb = cpool.tile([P, 3], F32)
            nc.sync.dma_start(wiT_sb[:], wiT[:])
            nc.sync.dma_start(whT_sb[:], whT[:])
            nc.sync.dma_start(wfcT_sb[:], wfcT[:])
            nc.sync.dma_start(kappa_sb[:], kappa[:])
            nc.sync.dma_start(smask_sb[:], smask[:])
            nc.sync.dma_start(rmask_sb[:], rmask[:])

            ones = cpool.tile([P, PCB], BF)
            nc.vector.memset(ones[:], 0.0)
            nc.vector.memset(ones[0:1, :], 1.0)
            zrow = cpool.tile([P, JT * PCB], BF)
            nc.vector.memset(zrow[:], 0.0)
            for slot in range(4):
                for g in range(2):
                    nc.sync.dma_start(cc_outA[slot][g], zrow[:])
                    nc.sync.dma_start(cc_outS[slot][g], zrow[:])

            def assemble(kk):
                """DMA + mask-combine the input chunk for iteration kk."""
                slot = kk % 4
                xb = io2.tile([P, IT, PCB], BF, tag="xb")
                nc.sync.dma_start(xb[:], xin[min(kk, PNCHUNK - 1)])
                ra = io2.tile([P, IT * PCB], BF, tag="ra")
                nc.sync.dma_start(ra[:], cc_outA[slot][0])
                rb = io2.tile([P, IT * PCB], BF, tag="rb")
                nc.sync.dma_start(rb[:], cc_outA[slot][1])
                rs = io2.tile([P, IT * PCB], BF, tag="rs")
                nc.sync.dma_start(rs[:], cc_outS[slot][0])
                v0 = io2.tile([P, IT * PCB], BF, tag="v0")
                nc.vector.tensor_scalar_mul(v0[:], ra[:], rmask_sb[:, 0:1])
                v1 = io2.tile([P, IT * PCB], BF, tag="v1")
                nc.vector.tensor_scalar_mul(v1[:], rb[:], rmask_sb[:, 1:2])
                t1 = io2.tile([P, IT * PCB], BF, tag="t1")
                nc.vector.tensor_tensor(t1[:], v0[:], v1[:], mybir.AluOpType.add)
                v2 = io2.tile([P, IT * PCB], BF, tag="v2")
                nc.vector.tensor_scalar_mul(v2[:], rs[:], rmask_sb[:, 2:3])
                t2 = io2.tile([P, IT * PCB], BF, tag="t2")
                nc.vector.tensor_tensor(t2[:], t1[:], v2[:], mybir.AluOpType.add)
                insb = io2.tile([P, IT, PCB], BF, tag="insb")
                nc.vector.tensor_tensor(
                    insb[:].rearrange("p i c -> p (i c)"), t2[:],
                    xb[:].rearrange("p i c -> p (i c)"), mybir.AluOpType.add)
                return insb

            def proj_mm(xps_ab, insb_t, pi):
                it, jt = pi // JT, pi % JT
                rhs = ones[:] if it == IT else insb_t[:, it, :]
                nc.tensor.matmul(
                    xps_ab[jt // 2][:, jt % 2, :, :].rearrange("p c b -> p (c b)"),
                    wiT_sb[:, (it * JT + jt) * P:(it * JT + jt + 1) * P],
                    rhs, start=(it == 0), stop=False, skip_group_check=True)

            NPROJ = (IT + 1) * JT        # 20
            PROJ0 = PCH - NPROJ          # interleave start step: 12

            with tc.tile_pool(name="xps", bufs=2, space="PSUM") as xpsp:
                # prologue: assemble + proj chunk 0.  Each jt-half gets its own
                # psum tile so the evac of one half overlaps the other half's
                # (and the next step's) matmuls instead of serializing.
                insb_cur = assemble(0)
                xa = xpsp.tile([P, 2, PCH, BL], F32, tag="xa")
                xb = xpsp.tile([P, 2, PCH, BL], F32, tag="xb")
                xps_next = (xa, xb)
                for pi in range(NPROJ):
                    proj_mm(xps_next, insb_cur, pi)
                insb_next = assemble(1)

                h_prev = hsq.tile([P, JT, PCH, BL], BF, tag="hseq")
                nc.vector.memset(h_prev[:], 0.0)

                for k in range(PNITER):
                    slot = k % 4
                    xps_cur = xps_next
                    insb_cur = insb_next
                    # assemble two chunks ahead so the DVE/DMA chain is off
                    # the critical path when proj(k+1) interleaves below
                    if k + 2 < PNITER:
                        insb_next = assemble(k + 2)
                    if k + 1 < PNITER:
                        xa = xpsp.tile([P, 2, PCH, BL], F32, tag="xa")
                        xb = xpsp.tile([P, 2, PCH, BL], F32, tag="xb")
                        xps_next = (xa, xb)
                    hg = io2.tile([P, KT, BL], BF, tag="hg")
                    nc.vector.tensor_scalar_mul(
                        hg[:], h_prev[:, :, PCH - 1, :], kappa_sb[:, k:k + 1])
                    hseq = hsq.tile([P, JT, PCH, BL], BF, tag="hseq")

                    for c in range(PCH):
                        for half in range(2):
                            xh = xps_cur[half]
                            for kt in range(KT):
                                rhs = (hg[:, kt, :] if c == 0
                                       else hseq[:, kt, c - 1, :])
                                for j2 in range(2):
                                    nc.tensor.matmul(
                                        xh[:, j2, c, :],
                                        whT_sb[:, (kt * JT + 2 * half + j2) * P:(kt * JT + 2 * half + j2 + 1) * P],
                                        rhs, start=False, stop=(kt == KT - 1),
                                        skip_group_check=True)
                            nc.vector.tensor_scalar_max(
                                hseq[:, 2 * half:2 * half + 2, c, :],
                                xh[:, :, c, :], 0.0)
                        if k + 1 < PNITER and c >= PROJ0:
                            proj_mm(xps_next, insb_cur, c - PROJ0)

                    # send + store
                    hflat = hseq[:].rearrange("p j c b -> p (j c b)")
                    sndA = io2.tile([P, JT * PCB], BF, tag="sndA")
                    nc.vector.tensor_scalar_mul(sndA[:], hflat, smask_sb[:, 0:1])
                    sndS = io2.tile([P, JT * PCB], BF, tag="sndS")
                    nc.vector.tensor_scalar_mul(sndS[:], hflat, smask_sb[:, 1:2])
                    nc.sync.dma_start(cc_inA[slot][:], sndA[:])
                    nc.sync.dma_start(cc_inS[slot][:], sndS[:])
                    nc.gpsimd.collective_compute(
                        "AllGather", mybir.AluOpType.bypass, PL_GROUPS_A,
                        ins=[cc_inA[slot][:]], outs=[cc_outA[slot][:]])
                    nc.gpsimd.collective_compute(
                        "AllGather", mybir.AluOpType.bypass, PL_GROUPS_S,
                        ins=[cc_inS[slot][:]], outs=[cc_outS[slot][:]])
                    nc.sync.dma_start(
                        hbuf[max(0, k - PDELAY * (L - 1))], hflat)
                    h_prev = hseq

            # fc tail over hbuf (only stage-3 cores' results are read)
            with (
                tc.tile_pool(name="psF", bufs=2, space="PSUM") as psF,
                tc.tile_pool(name="fio", bufs=2) as fio,
            ):
                for ch in range(PNCHUNK):
                    hch = fio.tile([P, JT, PCB], BF, tag="hch")
                    nc.sync.dma_start(
                        hch[:].rearrange("p j c -> p (j c)"), hbuf[ch])
                    osb = fio.tile([P, OT, PCB], F32, tag="osb")
                    for ot in range(OT):
                        fp = psF.tile([P, PCB], F32, tag="fp")
                        for ht in range(KT + 1):
                            rhs = ones[:] if ht == KT else hch[:, ht, :]
                            nc.tensor.matmul(
                                fp[:],
                                wfcT_sb[:, (ht * OT + ot) * P:(ht * OT + ot + 1) * P],
                                rhs, start=(ht == 0), stop=(ht == KT))
                        nc.vector.tensor_copy(osb[:, ot, :], fp[:])
                    nc.sync.dma_start(out[ch], osb[:])
    nc.compile()
    return nc


_NC_CACHE_PL = None


def _kernel_pl(x, W_ih, W_hh, b_ih, b_hh, W_fc, b_fc):
    global _NC_CACHE_PL

    def bias_block(bvec):
        """[512] -> [P, JT*P] block: row p==0 carries the bias, else 0."""
        blk = np.zeros((P, JT * P), np.float32)
        blk[0, :] = bvec
        return blk

    wfc_full = np.concatenate(
        [_pack_w(W_fc), bias_block(b_fc)], axis=1).astype(ml_dtypes.bfloat16)

    in_maps = []
    for core in range(NCORES):
        s = PL_STAGE[core]
        r = PL_REP[core]
        b0 = r * BL

        if s == 0:
            xh = x[b0:b0 + BL]
            xr = xh.reshape(BL, PNCHUNK, PCH, IT, P)
            xr = xr.transpose(1, 4, 3, 2, 0)
            xin = np.ascontiguousarray(
                xr.reshape(PNCHUNK, P, IT, PCB)).astype(ml_dtypes.bfloat16)
        else:
            xin = np.zeros((PNCHUNK, P, IT, PCB), ml_dtypes.bfloat16)

        wiT = np.concatenate(
            [_pack_w(W_ih[s]), bias_block(b_ih[s] + b_hh[s])],
            axis=1).astype(ml_dtypes.bfloat16)
        whT = _pack_w(W_hh[s]).astype(ml_dtypes.bfloat16)

        lo = PDELAY * s
        kap = np.zeros(PNITER, np.float32)
        kap[lo + 1:lo + PNCHUNK] = 1.0

        sm = np.array([1.0 if core in PL_SA else 0.0,
                       1.0 if core in PL_SS else 0.0], np.float32)
        rm = np.array([1.0 if core in PL_MA0 else 0.0,
                       1.0 if core in PL_MA1 else 0.0,
                       1.0 if core in PL_MS0 else 0.0], np.float32)

        in_maps.append({
            "xin": xin, "wiT": wiT, "whT": whT, "wfcT": wfc_full,
            "kappa": _bcast(kap), "smask": _bcast(sm), "rmask": _bcast(rm),
        })

    if _NC_CACHE_PL is None:
        _NC_CACHE_PL = _build_pl()
    global _last_in_maps, _last_nc
    _last_in_maps, _last_nc = in_maps, _NC_CACHE_PL
    res = run_bass_kernel_spmd(_NC_CACHE_PL, in_maps,
                               core_ids=list(range(NCORES)))

    y = np.empty((B, T, O), np.float32)
    for r in range(NREP):
        arr = res.results[PL_OUTCORE[r]]["out"]        # [NCHUNK, P, OT, CB]
        arr = arr.reshape(PNCHUNK, P, OT, PCH, BL)
        arr = arr.transpose(4, 0, 3, 2, 1)
        y[r * BL:(r + 1) * BL] = arr.reshape(BL, T, O)
    return y


import os

_VARIANT = os.environ.get("KVARIANT", "pl")
_DISPATCH = {"dp": _kernel_dp, "cc": _kernel_cc, "pl": _kernel_pl}


def kernel(x, W_ih, W_hh, b_ih, b_hh, W_fc, b_fc):
    x = np.asarray(x); W_ih = np.asarray(W_ih); W_hh = np.asarray(W_hh)
    b_ih = np.asarray(b_ih); b_hh = np.asarray(b_hh)
    W_fc = np.asarray(W_fc); b_fc = np.asarray(b_fc)
    return _DISPATCH[_VARIANT](x, W_ih, W_hh, b_ih, b_hh, W_fc, b_fc)



# revision 13
# speedup vs baseline: 1.0588x; 1.0588x over previous
"""DeepRNN (4-layer ReLU RNN, B=32 T=1024 H=512) on 8 trn2 NeuronCores.

Shipped strategy ("pl"): 4-stage layer pipeline x 2 batch replicas. Each core
owns one layer; chunks of 32 timesteps flow down the pipeline via masked pair
AllGathers (contiguous pairs + stride-4 pairs, the only group shapes the
runtime accepts), 3-deep slot parity. Per chunk, the input projection (bias folded in through a 5th
contraction block) is accumulated directly into PSUM and the recurrence
matmuls accumulate on top; the per-step relu evac is a single tensor_scalar
per jt-half with separate psum tiles per half so evac overlaps the other
half's matmuls. fc runs as a tail phase. Numerics: bf16 operands, fp32 PSUM
accumulate; rel err ~9e-3 vs fp32 reference.

Older variants kept for reference: "dp" (data-parallel, each core runs all 4
layers on B/8 lanes) and "cc" (earlier pipeline attempt; its group shapes
hang the runtime - do not use).
"""
import numpy as np
import ml_dtypes

import concourse.bass as bass
import concourse.bacc as bacc
import concourse.mybir as mybir
import concourse.tile as tile
from concourse.bass_utils import run_bass_kernel_spmd

# problem dims (hardcoded per contract)
B, T, H, O, L = 32, 1024, 512, 512, 4
P = 128
KT = JT = IT = OT = H // P          # 4 tiles per 512 dim
NCORES = 8
NREP = 2                            # batch replicas
BL = B // NREP                      # 16 batch lanes per core
CH = 32                             # timesteps per chunk
NCHUNK = T // CH                    # 32
DELAY = 2                           # iterations between produce and consume
NITER = NCHUNK + DELAY * (L - 1)    # 38
CB = CH * BL                        # 512 cols per chunk

BF = mybir.dt.bfloat16
F32 = mybir.dt.float32

GROUPS_A = [[0, 1], [2, 3], [4, 5], [6, 7]]
GROUPS_B = [[0, 7], [1, 2], [3, 4], [5, 6]]

# relu/gating DVE forms: "ts" uses tensor_scalar (1 sync-wait slot),
# "tt" uses tensor_tensor against a zero tile (2 wait slots).
USE_TT_FOR_MAX = True


def _build():
    nc = bacc.Bacc("TRN2", target_bir_lowering=False, debug=False, num_devices=8)
    xin = nc.declare_dram_parameter("xin", [NCHUNK, P, IT, CB], BF, isOutput=False)
    wiT = nc.declare_dram_parameter("wiT", [P, IT * JT * P], BF, isOutput=False)
    whT = nc.declare_dram_parameter("whT", [P, KT * JT * P], BF, isOutput=False)
    wfcT = nc.declare_dram_parameter("wfcT", [P, KT * OT * P], BF, isOutput=False)
    bias = nc.declare_dram_parameter("bias", [P, JT], F32, isOutput=False)
    bfc = nc.declare_dram_parameter("bfc", [P, OT], F32, isOutput=False)
    valid = nc.declare_dram_parameter("valid", [P, NITER], F32, isOutput=False)
    kappa = nc.declare_dram_parameter("kappa", [P, NITER], F32, isOutput=False)
    smask = nc.declare_dram_parameter("smask", [P, 2], F32, isOutput=False)
    out = nc.declare_dram_parameter("out", [NCHUNK + 1, P, OT, CB], F32, isOutput=True)

    # separate tensors per parity slot so Tile's name-level DRAM dependency
    # tracking doesn't serialize iteration k's consume behind iteration
    # k-1's collective (they touch different parities)
    cc_in = [nc.dram_tensor(f"cc_in{s}", [2, P, JT * CB], BF) for s in range(2)]
    cc_outA = [nc.dram_tensor(f"cc_outA{s}", [2, P, JT * CB], BF) for s in range(2)]
    cc_outB = [nc.dram_tensor(f"cc_outB{s}", [2, P, JT * CB], BF) for s in range(2)]

    with tile.TileContext(nc) as tc:
        with (
            tc.tile_pool(name="const", bufs=1) as cpool,
            tc.tile_pool(name="state", bufs=1) as spool,
            tc.tile_pool(name="io2", bufs=2) as io2,
            tc.tile_pool(name="hsq", bufs=2) as hsq,
            tc.tile_pool(name="step", bufs=2) as stp,
            tc.tile_pool(name="psP", bufs=2, space="PSUM") as psP,
            tc.tile_pool(name="psR", bufs=2, space="PSUM") as psR,
            tc.tile_pool(name="psF", bufs=2, space="PSUM") as psF,
        ):
            # ---- constants to SBUF, with wait-absorbing dummy reads ----
            wiT_sb = cpool.tile([P, IT * JT * P], BF)
            whT_sb = cpool.tile([P, KT * JT * P], BF)
            wfcT_sb = cpool.tile([P, KT * OT * P], BF)
            bias_sb = cpool.tile([P, JT], F32)
            bfc_sb = cpool.tile([P, OT], F32)
            valid_sb = cpool.tile([P, NITER], F32)
            kappa_sb = cpool.tile([P, NITER], F32)
            smask_sb = cpool.tile([P, 2], F32)
            nc.sync.dma_start(wiT_sb[:], wiT[:])
            nc.sync.dma_start(whT_sb[:], whT[:])
            nc.sync.dma_start(wfcT_sb[:], wfcT[:])
            nc.sync.dma_start(bias_sb[:], bias[:])
            nc.sync.dma_start(bfc_sb[:], bfc[:])
            nc.sync.dma_start(valid_sb[:], valid[:])
            nc.sync.dma_start(kappa_sb[:], kappa[:])
            nc.sync.dma_start(smask_sb[:], smask[:])
            dump = spool.tile([P, 8], F32)
            for i, t_ in enumerate(
                [wiT_sb, whT_sb, wfcT_sb, bias_sb, bfc_sb, valid_sb, kappa_sb,
                 smask_sb]):
                nc.vector.tensor_copy(dump[:, i:i + 1], t_[:, 0:1])

            zero_sb = cpool.tile([P, JT * BL], F32)
            nc.vector.memset(zero_sb[:], 0.0)
            zrow = cpool.tile([P, JT * CB], BF)
            nc.vector.memset(zrow[:], 0.0)
            # zero the collective landing zones (consumed at iters 0/1
            # before any collective has produced data)
            for slot in range(2):
                for g in range(2):
                    nc.sync.dma_start(cc_outA[slot][g], zrow[:])
                    nc.sync.dma_start(cc_outB[slot][g], zrow[:])

            # persistent recurrent state
            hcur = spool.tile([P, KT, BL], BF)
            nc.vector.memset(hcur[:], 0.0)

            for k in range(NITER):
                slot = k % 2
                # ---- assemble input chunk: x + recvA + recvB ----
                xb = io2.tile([P, IT, CB], BF, tag="xb")
                nc.sync.dma_start(xb[:], xin[min(k, NCHUNK - 1)])
                ra = io2.tile([P, IT * CB], BF, tag="ra")
                nc.sync.dma_start(ra[:], cc_outA[slot][0])
                rb = io2.tile([P, IT * CB], BF, tag="rb")
                nc.sync.dma_start(rb[:], cc_outB[slot][0])
                t1 = io2.tile([P, IT * CB], BF, tag="t1")
                nc.vector.tensor_tensor(
                    t1[:], xb[:].rearrange("p i c -> p (i c)"), ra[:],
                    mybir.AluOpType.add)
                insb = io2.tile([P, IT, CB], BF, tag="insb")
                nc.vector.tensor_tensor(
                    insb[:].rearrange("p i c -> p (i c)"), t1[:], rb[:],
                    mybir.AluOpType.add)

                # ---- input projection (gated by valid[k], bias folded) ----
                xw = io2.tile([P, JT, CH, BL], F32, tag="xw")
                for jt in range(JT):
                    pp = psP.tile([P, CB], F32, tag="pp")
                    for it in range(IT):
                        nc.tensor.matmul(
                            pp[:],
                            wiT_sb[:, (it * JT + jt) * P:(it * JT + jt + 1) * P],
                            insb[:, it, :],
                            start=(it == 0), stop=(it == IT - 1))
                    nc.vector.tensor_scalar(
                        xw[:, jt, :, :].rearrange("p c b -> p (c b)"), pp[:],
                        bias_sb[:, jt:jt + 1], valid_sb[:, k:k + 1],
                        mybir.AluOpType.add, mybir.AluOpType.mult)

                # ---- recurrence over the chunk ----
                hseq = hsq.tile([P, JT, CH, BL], BF, tag="hseq")
                nc.vector.tensor_scalar_mul(
                    hcur[:].rearrange("p k b -> p (k b)"),
                    hcur[:].rearrange("p k b -> p (k b)"),
                    kappa_sb[:, k:k + 1])
                for c in range(CH):
                    pt = psR.tile([P, JT * BL], F32, tag="pt")
                    for jt in range(JT):
                        for kt in range(KT):
                            rhs = (hcur[:, kt, :] if c == 0
                                   else hseq[:, kt, c - 1, :])
                            nc.tensor.matmul(
                                pt[:, jt * BL:(jt + 1) * BL],
                                whT_sb[:, (kt * JT + jt) * P:(kt * JT + jt + 1) * P],
                                rhs,
                                start=(kt == 0), stop=(kt == KT - 1))
                    tmp = stp.tile([P, JT, BL], F32, tag="tmp")
                    nc.vector.tensor_tensor(
                        tmp[:], pt[:].rearrange("p (j b) -> p j b", j=JT),
                        xw[:, :, c, :], mybir.AluOpType.add)
                    dst = hseq[:, :, c, :]
                    if USE_TT_FOR_MAX:
                        nc.vector.tensor_tensor(
                            dst, tmp[:],
                            zero_sb[:].rearrange("p (j b) -> p j b", j=JT),
                            mybir.AluOpType.max)
                    else:
                        nc.vector.tensor_scalar_max(dst, tmp[:], 0.0)
                nc.vector.tensor_copy(hcur[:], hseq[:, :, CH - 1, :])

                # ---- fc head (only stage-3 cores' results are kept) ----
                osb = io2.tile([P, OT, CB], F32, tag="osb")
                for ot in range(OT):
                    fp = psF.tile([P, CB], F32, tag="fp")
                    for ht in range(KT):
                        nc.tensor.matmul(
                            fp[:],
                            wfcT_sb[:, (ht * OT + ot) * P:(ht * OT + ot + 1) * P],
                            hseq[:, ht, :, :].rearrange("p c b -> p (c b)"),
                            start=(ht == 0), stop=(ht == KT - 1))
                    nc.vector.tensor_scalar_add(
                        osb[:, ot, :], fp[:], bfc_sb[:, ot:ot + 1])
                oslice = k - DELAY * (L - 1)
                if not (0 <= oslice < NCHUNK):
                    oslice = NCHUNK
                nc.sync.dma_start(out[oslice], osb[:])

                # ---- send my chunk to successor (masked 2-slot write) ----
                snd = io2.tile([P, 2, JT * CB], BF, tag="snd")
                hflat = hseq[:].rearrange("p j c b -> p (j c b)")
                nc.vector.tensor_scalar_mul(snd[:, 0, :], hflat, smask_sb[:, 0:1])
                nc.vector.tensor_scalar_mul(snd[:, 1, :], hflat, smask_sb[:, 1:2])
                nc.sync.dma_start(cc_in[slot][0], snd[:, 0, :])
                nc.sync.dma_start(cc_in[slot][1], snd[:, 1, :])
                nc.gpsimd.collective_compute(
                    "AllGather", mybir.AluOpType.bypass, GROUPS_A,
                    ins=[cc_in[slot][0]], outs=[cc_outA[slot][:]])
                nc.gpsimd.collective_compute(
                    "AllGather", mybir.AluOpType.bypass, GROUPS_B,
                    ins=[cc_in[slot][1]], outs=[cc_outB[slot][:]])
    nc.compile()
    return nc


BL8 = B // NCORES                    # 4 batch lanes per core in DP mode
TB = T * BL8                         # 4096 cols


def _build_dp():
    """Data-parallel fallback: each core runs all 4 layers + fc on B/8=4
    batch lanes. No collectives."""
    nc = bacc.Bacc("TRN2", target_bir_lowering=False, debug=False,
                   num_devices=8)
    xin = nc.declare_dram_parameter("xin", [P, IT, TB], BF, isOutput=False)
    wiT = nc.declare_dram_parameter("wiT", [P, L * IT * JT * P], BF, isOutput=False)
    whT = nc.declare_dram_parameter("whT", [P, L * KT * JT * P], BF, isOutput=False)
    wfcT = nc.declare_dram_parameter("wfcT", [P, KT * OT * P], BF, isOutput=False)
    bias = nc.declare_dram_parameter("bias", [P, L * JT], F32, isOutput=False)
    bfc = nc.declare_dram_parameter("bfc", [P, OT], F32, isOutput=False)
    out = nc.declare_dram_parameter("out", [P, OT, TB], F32, isOutput=True)

    NCOL = 512                       # psum bank limit: 512 fp32 out cols
    NSPL = TB // NCOL                # 4

    with tile.TileContext(nc) as tc:
        with (
            tc.tile_pool(name="const", bufs=1) as cpool,
            tc.tile_pool(name="act", bufs=1) as apool,
            tc.tile_pool(name="step", bufs=2) as stp,
            tc.tile_pool(name="psP", bufs=2, space="PSUM") as psP,
            tc.tile_pool(name="psR", bufs=2, space="PSUM") as psR,
        ):
            wiT_sb = cpool.tile([P, L * IT * JT * P], BF)
            whT_sb = cpool.tile([P, L * KT * JT * P], BF)
            wfcT_sb = cpool.tile([P, KT * OT * P], BF)
            bias_sb = cpool.tile([P, L * JT], F32)
            bfc_sb = cpool.tile([P, OT], F32)
            nc.sync.dma_start(wiT_sb[:], wiT[:])
            nc.sync.dma_start(whT_sb[:], whT[:])
            nc.sync.dma_start(wfcT_sb[:], wfcT[:])
            nc.sync.dma_start(bias_sb[:], bias[:])
            nc.sync.dma_start(bfc_sb[:], bfc[:])
            zero_sb = cpool.tile([P, JT, BL8], F32)
            nc.vector.memset(zero_sb[:], 0.0)
            zbf = cpool.tile([P, KT, BL8], BF)
            nc.vector.memset(zbf[:], 0.0)

            cur = apool.tile([P, IT, TB], BF, tag="cur")
            nc.sync.dma_start(cur[:], xin[:])

            for l in range(L):
                # input projection for the whole sequence
                xw = apool.tile([P, JT, T, BL8], BF, tag=f"xw{l % 2}")
                cur_f = (cur[:].rearrange("p i t -> p (i t)") if l == 0
                         else cur[:].rearrange("p j t b -> p (j t b)"))
                for jt in range(JT):
                    for sp in range(NSPL):
                        pp = psP.tile([P, NCOL], F32, tag="pp")
                        for it in range(IT):
                            nc.tensor.matmul(
                                pp[:],
                                wiT_sb[:, (l * 16 + it * JT + jt) * P:(l * 16 + it * JT + jt + 1) * P],
                                cur_f[:, it * TB + sp * NCOL:it * TB + (sp + 1) * NCOL],
                                start=(it == 0), stop=(it == IT - 1))
                        xw_f = xw[:].rearrange("p j t b -> p (j t b)")
                        nc.vector.tensor_scalar_add(
                            xw_f[:, jt * TB + sp * NCOL:jt * TB + (sp + 1) * NCOL],
                            pp[:], bias_sb[:, l * JT + jt:l * JT + jt + 1])
                # recurrence
                hseq = apool.tile([P, JT, T, BL8], BF, tag=f"hs{l % 2}")
                for t in range(T):
                    pt = psR.tile([P, JT * BL8], F32, tag="pt")
                    for jt in range(JT):
                        for kt in range(KT):
                            nc.tensor.matmul(
                                pt[:, jt * BL8:(jt + 1) * BL8],
                                whT_sb[:, (l * 16 + kt * JT + jt) * P:(l * 16 + kt * JT + jt + 1) * P],
                                hseq[:, kt, t - 1, :] if t > 0 else zbf[:, kt, :],
                                start=(kt == 0), stop=(kt == KT - 1))
                    tmp = stp.tile([P, JT, BL8], F32, tag="tmp")
                    nc.vector.tensor_tensor(
                        tmp[:], pt[:].rearrange("p (j b) -> p j b", j=JT),
                        xw[:, :, t, :], mybir.AluOpType.add)
                    nc.vector.tensor_tensor(
                        hseq[:, :, t, :], tmp[:], zero_sb[:],
                        mybir.AluOpType.max)
                cur = hseq

            # fc head
            for ot in range(OT):
                for sp in range(NSPL):
                    fp = psP.tile([P, NCOL], F32, tag="pp")
                    cur_f = cur[:].rearrange("p h t b -> p (h t b)")
                    for ht in range(KT):
                        nc.tensor.matmul(
                            fp[:],
                            wfcT_sb[:, (ht * OT + ot) * P:(ht * OT + ot + 1) * P],
                            cur_f[:, ht * TB + sp * NCOL:ht * TB + (sp + 1) * NCOL],
                            start=(ht == 0), stop=(ht == KT - 1))
                    osb = stp.tile([P, NCOL], F32, tag="osb")
                    nc.vector.tensor_scalar_add(
                        osb[:], fp[:], bfc_sb[:, ot:ot + 1])
                    nc.sync.dma_start(
                        out[:, ot, sp * NCOL:(sp + 1) * NCOL], osb[:])
    nc.compile()
    return nc


CH2 = 32                             # rec chunk (psum-resident xw)


def _build_dp2():
    """DP with xw kept in PSUM: proj writes psum, rec matmuls accumulate
    on top (start=False), relu+bias evac is one tensor_scalar per jt."""
    nc = bacc.Bacc("TRN2", target_bir_lowering=False, debug=False,
                   num_devices=8)
    xin = nc.declare_dram_parameter("xin", [P, IT, TB], BF, isOutput=False)
    wiT = nc.declare_dram_parameter("wiT", [P, L * IT * JT * P], BF, isOutput=False)
    whT = nc.declare_dram_parameter("whT", [P, L * KT * JT * P], BF, isOutput=False)
    wfcT = nc.declare_dram_parameter("wfcT", [P, KT * OT * P], BF, isOutput=False)
    bias = nc.declare_dram_parameter("bias", [P, L * JT], F32, isOutput=False)
    bfc = nc.declare_dram_parameter("bfc", [P, OT], F32, isOutput=False)
    out = nc.declare_dram_parameter("out", [P, OT, TB], F32, isOutput=True)

    NCOL = 512
    NSPL = TB // NCOL
    NCH = T // CH2

    with tile.TileContext(nc) as tc:
        with (
            tc.tile_pool(name="const", bufs=1) as cpool,
            tc.tile_pool(name="act", bufs=1) as apool,
            tc.tile_pool(name="step", bufs=2) as stp,
            tc.tile_pool(name="psP", bufs=2, space="PSUM") as psP,
            tc.tile_pool(name="psX", bufs=2, space="PSUM") as psX,
        ):
            wiT_sb = cpool.tile([P, L * IT * JT * P], BF)
            whT_sb = cpool.tile([P, L * KT * JT * P], BF)
            wfcT_sb = cpool.tile([P, KT * OT * P], BF)
            bias_sb = cpool.tile([P, L * JT], F32)
            bfc_sb = cpool.tile([P, OT], F32)
            nc.sync.dma_start(wiT_sb[:], wiT[:])
            nc.sync.dma_start(whT_sb[:], whT[:])
            nc.sync.dma_start(wfcT_sb[:], wfcT[:])
            nc.sync.dma_start(bias_sb[:], bias[:])
            nc.sync.dma_start(bfc_sb[:], bfc[:])
            zbf = cpool.tile([P, KT, BL8], BF)
            nc.vector.memset(zbf[:], 0.0)

            cur = apool.tile([P, IT, TB], BF, tag="cur")
            nc.sync.dma_start(cur[:], xin[:])

            for l in range(L):
                cur_f = (cur[:].rearrange("p i t -> p (i t)") if l == 0
                         else cur[:].rearrange("p j t b -> p (j t b)"))
                hseq = apool.tile([P, JT, T, BL8], BF, tag=f"hs{l % 2}")
                for kc in range(NCH):
                    # proj chunk directly into psum [P, (jt, c, b)]
                    xps = psX.tile([P, JT, CH2, BL8], F32, tag="xps")
                    cb0 = kc * CH2 * BL8
                    for jt in range(JT):
                        for it in range(IT):
                            nc.tensor.matmul(
                                xps[:, jt, :, :],
                                wiT_sb[:, (l * 16 + it * JT + jt) * P:
                                       (l * 16 + it * JT + jt + 1) * P],
                                cur_f[:, it * TB + cb0:it * TB + cb0 + CH2 * BL8],
                                start=(it == 0), stop=False,
                                skip_group_check=True)
                    # recurrence, accumulating onto the psum xw
                    for c in range(CH2):
                        t = kc * CH2 + c
                        for jt in range(JT):
                            for kt in range(KT):
                                rhs = (hseq[:, kt, t - 1, :] if t > 0
                                       else zbf[:, kt, :])
                                nc.tensor.matmul(
                                    xps[:, jt, c, :],
                                    whT_sb[:, (l * 16 + kt * JT + jt) * P:
                                           (l * 16 + kt * JT + jt + 1) * P],
                                    rhs,
                                    start=False, stop=(kt == KT - 1),
                                    skip_group_check=True)
                            nc.vector.tensor_scalar(
                                hseq[:, jt, t, :], xps[:, jt, c, :],
                                bias_sb[:, l * JT + jt:l * JT + jt + 1], 0.0,
                                mybir.AluOpType.add, mybir.AluOpType.max)
                cur = hseq

            for ot in range(OT):
                cur_f = cur[:].rearrange("p h t b -> p (h t b)")
                for sp in range(NSPL):
                    fp = psP.tile([P, NCOL], F32, tag="pp")
                    for ht in range(KT):
                        nc.tensor.matmul(
                            fp[:],
                            wfcT_sb[:, (ht * OT + ot) * P:(ht * OT + ot + 1) * P],
                            cur_f[:, ht * TB + sp * NCOL:ht * TB + (sp + 1) * NCOL],
                            start=(ht == 0), stop=(ht == KT - 1))
                    osb = stp.tile([P, NCOL], F32, tag="osb")
                    nc.vector.tensor_scalar_add(
                        osb[:], fp[:], bfc_sb[:, ot:ot + 1])
                    nc.sync.dma_start(
                        out[:, ot, sp * NCOL:(sp + 1) * NCOL], osb[:])
    nc.compile()
    return nc


def _kernel_dp(x, W_ih, W_hh, b_ih, b_hh, W_fc, b_fc):
    global _NC_CACHE_DP
    in_maps = []
    wiT = np.concatenate([_pack_w(W_ih[l]) for l in range(L)], axis=1).astype(ml_dtypes.bfloat16)
    whT = np.concatenate([_pack_w(W_hh[l]) for l in range(L)], axis=1).astype(ml_dtypes.bfloat16)
    wfcT = _pack_w(W_fc).astype(ml_dtypes.bfloat16)
    bias = np.ascontiguousarray(
        (b_ih + b_hh).reshape(L * JT, P).T).astype(np.float32)
    bfc_a = np.ascontiguousarray(b_fc.reshape(OT, P).T).astype(np.float32)
    for core in range(NCORES):
        b0 = core * BL8
        xh = x[b0:b0 + BL8]                           # [BL8, T, H]
        xr = xh.reshape(BL8, T, IT, P).transpose(3, 2, 1, 0)   # p it t b
        xin = np.ascontiguousarray(
            xr.reshape(P, IT, TB)).astype(ml_dtypes.bfloat16)
        in_maps.append({"xin": xin, "wiT": wiT, "whT": whT, "wfcT": wfcT,
                        "bias": bias, "bfc": bfc_a})
    if _NC_CACHE_DP is None:
        _NC_CACHE_DP = _build_dp()
    global _last_in_maps, _last_nc
    _last_in_maps, _last_nc = in_maps, _NC_CACHE_DP
    res = run_bass_kernel_spmd(_NC_CACHE_DP, in_maps,
                               core_ids=list(range(NCORES)))
    y = np.empty((B, T, O), np.float32)
    for core in range(NCORES):
        arr = res.results[core]["out"]                # [P, OT, TB]
        arr = arr.reshape(P, OT, T, BL8).transpose(3, 2, 1, 0)
        y[core * BL8:(core + 1) * BL8] = arr.reshape(BL8, T, O)
    return y


_NC_CACHE_DP = None


def _pack_w(w):
    """[512(out j), 512(in k)] -> [P, (kt*JT+jt)*P + j] = w[jt*P+j, kt*P+p]"""
    r = w.reshape(JT, P, KT, P)            # [jt, j, kt, p]
    r = r.transpose(3, 2, 0, 1)            # [p, kt, jt, j]
    return np.ascontiguousarray(r.reshape(P, KT * JT * P))


def _bcast(v):
    return np.ascontiguousarray(np.broadcast_to(v, (P,) + v.shape).astype(np.float32))


_NC_CACHE = None


def _kernel_cc(x, W_ih, W_hh, b_ih, b_hh, W_fc, b_fc):
    global _NC_CACHE
    x = np.asarray(x); W_ih = np.asarray(W_ih); W_hh = np.asarray(W_hh)
    b_ih = np.asarray(b_ih); b_hh = np.asarray(b_hh)
    W_fc = np.asarray(W_fc); b_fc = np.asarray(b_fc)

    in_maps = []
    for core in range(NCORES):
        s = core % 4          # pipeline stage == layer index
        rep = core // 4       # batch replica
        b0 = rep * BL

        if s == 0:
            xh = x[b0:b0 + BL]                                   # [BL, T, H]
            xr = xh.reshape(BL, NCHUNK, CH, IT, P)               # b k c i p
            xr = xr.transpose(1, 4, 3, 2, 0)                     # k p i c b
            xin = np.ascontiguousarray(
                xr.reshape(NCHUNK, P, IT, CB)).astype(ml_dtypes.bfloat16)
        else:
            xin = np.zeros((NCHUNK, P, IT, CB), ml_dtypes.bfloat16)

        wiT = _pack_w(W_ih[s]).astype(ml_dtypes.bfloat16)
        whT = _pack_w(W_hh[s]).astype(ml_dtypes.bfloat16)
        wfcT = _pack_w(W_fc).astype(ml_dtypes.bfloat16)
        # bias[j] with j = jt*P + p  ->  [P, JT] element [p, jt]
        bias = np.ascontiguousarray(
            (b_ih[s] + b_hh[s]).reshape(JT, P).T).astype(np.float32)
        bfc_a = np.ascontiguousarray(b_fc.reshape(OT, P).T).astype(np.float32)

        val = np.zeros(NITER, np.float32)
        kap = np.zeros(NITER, np.float32)
        lo = DELAY * s
        val[lo:lo + NCHUNK] = 1.0
        kap[lo + 1:lo + NCHUNK] = 1.0
        succ = (core + 1) % NCORES
        sA = 1.0 if (core % 2 == 0 and succ % 4 != 0) else 0.0
        sB = 1.0 if (core % 2 == 1 and succ % 4 != 0) else 0.0

        in_maps.append({
            "xin": xin,
            "wiT": wiT, "whT": whT, "wfcT": wfcT,
            "bias": bias,
            "bfc": bfc_a,
            "valid": _bcast(val),
            "kappa": _bcast(kap),
            "smask": _bcast(np.array([sA, sB], np.float32)),
        })

    global _last_in_maps
    _last_in_maps = in_maps
    if _NC_CACHE is None:
        _NC_CACHE = _build()
    res = run_bass_kernel_spmd(_NC_CACHE, in_maps, core_ids=list(range(NCORES)))

    y = np.empty((B, T, O), np.float32)
    for rep in range(NREP):
        arr = res.results[3 + 4 * rep]["out"][:NCHUNK]       # [k, P, OT, CB]
        arr = arr.reshape(NCHUNK, P, OT, CH, BL)
        arr = arr.transpose(4, 0, 3, 2, 1)                   # b k c ot p
        y[rep * BL:(rep + 1) * BL] = arr.reshape(BL, T, O)
    return y


def _bcast_rows(a):
    """per-partition table already [P, ...] -> passthrough; 1d -> broadcast"""
    a = np.asarray(a, np.float32)
    if a.ndim == 1:
        return _bcast(a)
    return np.ascontiguousarray(a)


# ---------------------------------------------------------------------------
# pl: 4-stage layer pipeline x 2 replicas with pair-collective handoff.
#
# Chains: 0 -A-> 1 -S-> 5 -A-> 4   and   2 -A-> 3 -S-> 7 -A-> 6
# (A = contiguous pairs [[0,1],[2,3],[4,5],[6,7]],
#  S = stride-4 pairs  [[0,4],[1,5],[2,6],[3,7]] — the only group shapes the
#  runtime accepts; sends and receives are masked per-core via dram data so
#  the SPMD instruction stream stays uniform.)
#
# Each iteration k: rec(chunk k) on psum buffer k%2 (xw pre-accumulated by
# proj during iter k-1, bias folded in via a 5th contraction block), proj for
# chunk k+1 interleaved into the late rec steps, hseq chunk sent to the
# successor stage via 2 pair AllGathers with 3-deep slot parity (DELAY=3).
# fc runs as a tail phase over hbuf after the pipeline drains.
# ---------------------------------------------------------------------------
PCH = 32                                # pl: timesteps per chunk
PNCHUNK = T // PCH                      # 32
PDELAY = 3
PNITER = PNCHUNK + PDELAY * (L - 1)     # 41
PCB = PCH * BL                          # 512 cols per chunk

PL_STAGE = {0: 0, 1: 1, 5: 2, 4: 3, 2: 0, 3: 1, 7: 2, 6: 3}
PL_REP = {0: 0, 1: 0, 5: 0, 4: 0, 2: 1, 3: 1, 7: 1, 6: 1}
PL_GROUPS_A = [[0, 1], [2, 3], [4, 5], [6, 7]]
PL_GROUPS_S = [[0, 4], [1, 5], [2, 6], [3, 7]]
PL_SA = (0, 2, 5, 7)                    # A-collective senders
PL_SS = (1, 3)                          # S-collective senders
PL_MA0 = (1, 3)                         # read A member 0 (pred is lower pair member)
PL_MA1 = (4, 6)                         # read A member 1
PL_MS0 = (5, 7)                         # read S member 0
PL_OUTCORE = {0: 4, 1: 6}               # replica -> stage-3 core


def _build_pl():
    nc = bacc.Bacc("TRN2", target_bir_lowering=False, debug=False,
                   num_devices=8)
    xin = nc.declare_dram_parameter("xin", [PNCHUNK, P, IT, PCB], BF, isOutput=False)
    wiT = nc.declare_dram_parameter("wiT", [P, (IT + 1) * JT * P], BF, isOutput=False)
    whT = nc.declare_dram_parameter("whT", [P, KT * JT * P], BF, isOutput=False)
    wfcT = nc.declare_dram_parameter("wfcT", [P, (KT + 1) * OT * P], BF, isOutput=False)
    kappa = nc.declare_dram_parameter("kappa", [P, PNITER], F32, isOutput=False)
    smask = nc.declare_dram_parameter("smask", [P, 2], F32, isOutput=False)
    rmask = nc.declare_dram_parameter("rmask", [P, 3], F32, isOutput=False)
    out = nc.declare_dram_parameter("out", [PNCHUNK, P, OT, PCB], F32, isOutput=True)

    hbuf = nc.dram_tensor("hbuf", [PNCHUNK, P, JT * PCB], BF)
    cc_inA = [nc.dram_tensor(f"cc_inA{s}", [P, JT * PCB], BF) for s in range(3)]
    cc_inS = [nc.dram_tensor(f"cc_inS{s}", [P, JT * PCB], BF) for s in range(3)]
    cc_outA = [nc.dram_tensor(f"cc_outA{s}", [2, P, JT * PCB], BF) for s in range(3)]
    cc_outS = [nc.dram_tensor(f"cc_outS{s}", [2, P, JT * PCB], BF) for s in range(3)]

    with tile.TileContext(nc) as tc:
        with (
            tc.tile_pool(name="const", bufs=1) as cpool,
            tc.tile_pool(name="io2", bufs=2) as io2,
            tc.tile_pool(name="hsq", bufs=2) as hsq,
        ):
            # constants
            wiT_sb = cpool.tile([P, (IT + 1) * JT * P], BF)
            whT_sb = cpool.tile([P, KT * JT * P], BF)
            wfcT_sb = cpool.tile([P, (KT + 1) * OT * P], BF)
            kappa_sb = cpool.tile([P, PNITER], F32)
            smask_sb = cpool.tile([P, 2], F32)
            rmask_sb = cpool.tile([P, 3], F32)
            nc.sync.dma_start(wiT_sb[:], wiT[:])
            nc.sync.dma_start(whT_sb[:], whT[:])
            nc.sync.dma_start(wfcT_sb[:], wfcT[:])
            nc.sync.dma_start(kappa_sb[:], kappa[:])
            nc.sync.dma_start(smask_sb[:], smask[:])
            nc.sync.dma_start(rmask_sb[:], rmask[:])

            ones = cpool.tile([P, PCB], BF)
            nc.vector.memset(ones[:], 0.0)
            nc.vector.memset(ones[0:1, :], 1.0)
            zrow = cpool.tile([P, JT * PCB], BF)
            nc.vector.memset(zrow[:], 0.0)
            for slot in range(3):
                for g in range(2):
                    nc.sync.dma_start(cc_outA[slot][g], zrow[:])
                    nc.sync.dma_start(cc_outS[slot][g], zrow[:])

            def assemble(kk):
                """DMA + mask-combine the input chunk for iteration kk."""
                slot = kk % 3
                xb = io2.tile([P, IT, PCB], BF, tag="xb")
                nc.sync.dma_start(xb[:], xin[min(kk, PNCHUNK - 1)])
                ra = io2.tile([P, IT * PCB], BF, tag="ra")
                nc.sync.dma_start(ra[:], cc_outA[slot][0])
                rb = io2.tile([P, IT * PCB], BF, tag="rb")
                nc.sync.dma_start(rb[:], cc_outA[slot][1])
                rs = io2.tile([P, IT * PCB], BF, tag="rs")
                nc.sync.dma_start(rs[:], cc_outS[slot][0])
                v0 = io2.tile([P, IT * PCB], BF, tag="v0")
                nc.vector.tensor_scalar_mul(v0[:], ra[:], rmask_sb[:, 0:1])
                v1 = io2.tile([P, IT * PCB], BF, tag="v1")
                nc.vector.tensor_scalar_mul(v1[:], rb[:], rmask_sb[:, 1:2])
                t1 = io2.tile([P, IT * PCB], BF, tag="t1")
                nc.vector.tensor_tensor(t1[:], v0[:], v1[:], mybir.AluOpType.add)
                v2 = io2.tile([P, IT * PCB], BF, tag="v2")
                nc.vector.tensor_scalar_mul(v2[:], rs[:], rmask_sb[:, 2:3])
                t2 = io2.tile([P, IT * PCB], BF, tag="t2")
                nc.vector.tensor_tensor(t2[:], t1[:], v2[:], mybir.AluOpType.add)
                insb = io2.tile([P, IT, PCB], BF, tag="insb")
                nc.vector.tensor_tensor(
                    insb[:].rearrange("p i c -> p (i c)"), t2[:],
                    xb[:].rearrange("p i c -> p (i c)"), mybir.AluOpType.add)
                return insb

            def proj_mm(xps_ab, insb_t, pi):
                it, jt = pi // JT, pi % JT
                rhs = ones[:] if it == IT else insb_t[:, it, :]
                nc.tensor.matmul(
                    xps_ab[jt // 2][:, jt % 2, :, :].rearrange("p c b -> p (c b)"),
                    wiT_sb[:, (it * JT + jt) * P:(it * JT + jt + 1) * P],
                    rhs, start=(it == 0), stop=False, skip_group_check=True)

            NPROJ = (IT + 1) * JT        # 20
            PROJ0 = PCH - NPROJ          # interleave start step: 12

            with tc.tile_pool(name="xps", bufs=2, space="PSUM") as xpsp:
                # prologue: assemble + proj chunk 0.  Each jt-half gets its own
                # psum tile so the evac of one half overlaps the other half's
                # (and the next step's) matmuls instead of serializing.
                insb_cur = assemble(0)
                xa = xpsp.tile([P, 2, PCH, BL], F32, tag="xa")
                xb = xpsp.tile([P, 2, PCH, BL], F32, tag="xb")
                xps_next = (xa, xb)
                for pi in range(NPROJ):
                    proj_mm(xps_next, insb_cur, pi)
                insb_next = assemble(1)

                h_prev = hsq.tile([P, JT, PCH, BL], BF, tag="hseq")
                nc.vector.memset(h_prev[:], 0.0)

                for k in range(PNITER):
                    slot = k % 3
                    xps_cur = xps_next
                    insb_cur = insb_next
                    # assemble two chunks ahead so the DVE/DMA chain is off
                    # the critical path when proj(k+1) interleaves below
                    if k + 2 < PNITER:
                        insb_next = assemble(k + 2)
                    if k + 1 < PNITER:
                        xa = xpsp.tile([P, 2, PCH, BL], F32, tag="xa")
                        xb = xpsp.tile([P, 2, PCH, BL], F32, tag="xb")
                        xps_next = (xa, xb)
                    hg = io2.tile([P, KT, BL], BF, tag="hg")
                    nc.vector.tensor_scalar_mul(
                        hg[:], h_prev[:, :, PCH - 1, :], kappa_sb[:, k:k + 1])
                    hseq = hsq.tile([P, JT, PCH, BL], BF, tag="hseq")

                    for c in range(PCH):
                        for half in range(2):
                            xh = xps_cur[half]
                            for kt in range(KT):
                                rhs = (hg[:, kt, :] if c == 0
                                       else hseq[:, kt, c - 1, :])
                                for j2 in range(2):
                                    nc.tensor.matmul(
                                        xh[:, j2, c, :],
                                        whT_sb[:, (kt * JT + 2 * half + j2) * P:(kt * JT + 2 * half + j2 + 1) * P],
                                        rhs, start=False, stop=(kt == KT - 1),
                                        skip_group_check=True)
                            nc.vector.tensor_scalar_max(
                                hseq[:, 2 * half:2 * half + 2, c, :],
                                xh[:, :, c, :], 0.0)
                        if k + 1 < PNITER and c >= PROJ0:
                            proj_mm(xps_next, insb_cur, c - PROJ0)

                    # send + store
                    hflat = hseq[:].rearrange("p j c b -> p (j c b)")
                    sndA = io2.tile([P, JT * PCB], BF, tag="sndA")
                    nc.vector.tensor_scalar_mul(sndA[:], hflat, smask_sb[:, 0:1])
                    sndS = io2.tile([P, JT * PCB], BF, tag="sndS")
                    nc.vector.tensor_scalar_mul(sndS[:], hflat, smask_sb[:, 1:2])
                    nc.sync.dma_start(cc_inA[slot][:], sndA[:])
                    nc.sync.dma_start(cc_inS[slot][:], sndS[:])
                    nc.gpsimd.collective_compute(
                        "AllGather", mybir.AluOpType.bypass, PL_GROUPS_A,
                        ins=[cc_inA[slot][:]], outs=[cc_outA[slot][:]])
                    nc.gpsimd.collective_compute(
                        "AllGather", mybir.AluOpType.bypass, PL_GROUPS_S,
                        ins=[cc_inS[slot][:]], outs=[cc_outS[slot][:]])
                    nc.sync.dma_start(
                        hbuf[max(0, k - PDELAY * (L - 1))], hflat)
                    h_prev = hseq

            # fc tail over hbuf (only stage-3 cores' results are read)
            with (
                tc.tile_pool(name="psF", bufs=2, space="PSUM") as psF,
                tc.tile_pool(name="fio", bufs=2) as fio,
            ):
                for ch in range(PNCHUNK):
                    hch = fio.tile([P, JT, PCB], BF, tag="hch")
                    nc.sync.dma_start(
                        hch[:].rearrange("p j c -> p (j c)"), hbuf[ch])
                    osb = fio.tile([P, OT, PCB], F32, tag="osb")
                    for ot in range(OT):
                        fp = psF.tile([P, PCB], F32, tag="fp")
                        for ht in range(KT + 1):
                            rhs = ones[:] if ht == KT else hch[:, ht, :]
                            nc.tensor.matmul(
                                fp[:],
                                wfcT_sb[:, (ht * OT + ot) * P:(ht * OT + ot + 1) * P],
                                rhs, start=(ht == 0), stop=(ht == KT))
                        nc.vector.tensor_copy(osb[:, ot, :], fp[:])
                    nc.sync.dma_start(out[ch], osb[:])
    nc.compile()
    return nc


_NC_CACHE_PL = None


def _kernel_pl(x, W_ih, W_hh, b_ih, b_hh, W_fc, b_fc):
    global _NC_CACHE_PL

    def bias_block(bvec):
        """[512] -> [P, JT*P] block: row p==0 carries the bias, else 0."""
        blk = np.zeros((P, JT * P), np.float32)
        blk[0, :] = bvec
        return blk

    wfc_full = np.concatenate(
        [_pack_w(W_fc), bias_block(b_fc)], axis=1).astype(ml_dtypes.bfloat16)

    in_maps = []
    for core in range(NCORES):
        s = PL_STAGE[core]
        r = PL_REP[core]
        b0 = r * BL

        if s == 0:
            xh = x[b0:b0 + BL]
            xr = xh.reshape(BL, PNCHUNK, PCH, IT, P)
            xr = xr.transpose(1, 4, 3, 2, 0)
            xin = np.ascontiguousarray(
                xr.reshape(PNCHUNK, P, IT, PCB)).astype(ml_dtypes.bfloat16)
        else:
            xin = np.zeros((PNCHUNK, P, IT, PCB), ml_dtypes.bfloat16)

        wiT = np.concatenate(
            [_pack_w(W_ih[s]), bias_block(b_ih[s] + b_hh[s])],
            axis=1).astype(ml_dtypes.bfloat16)
        whT = _pack_w(W_hh[s]).astype(ml_dtypes.bfloat16)

        lo = PDELAY * s
        kap = np.zeros(PNITER, np.float32)
        kap[lo + 1:lo + PNCHUNK] = 1.0

        sm = np.array([1.0 if core in PL_SA else 0.0,
                       1.0 if core in PL_SS else 0.0], np.float32)
        rm = np.array([1.0 if core in PL_MA0 else 0.0,
                       1.0 if core in PL_MA1 else 0.0,
                       1.0 if core in PL_MS0 else 0.0], np.float32)

        in_maps.append({
            "xin": xin, "wiT": wiT, "whT": whT, "wfcT": wfc_full,
            "kappa": _bcast(kap), "smask": _bcast(sm), "rmask": _bcast(rm),
        })

    if _NC_CACHE_PL is None:
        _NC_CACHE_PL = _build_pl()
    global _last_in_maps, _last_nc
    _last_in_maps, _last_nc = in_maps, _NC_CACHE_PL
    res = run_bass_kernel_spmd(_NC_CACHE_PL, in_maps,
                               core_ids=list(range(NCORES)))

    y = np.empty((B, T, O), np.float32)
    for r in range(NREP):
        arr = res.results[PL_OUTCORE[r]]["out"]        # [NCHUNK, P, OT, CB]
        arr = arr.reshape(PNCHUNK, P, OT, PCH, BL)
        arr = arr.transpose(4, 0, 3, 2, 1)
        y[r * BL:(r + 1) * BL] = arr.reshape(BL, T, O)
    return y


import os

_VARIANT = os.environ.get("KVARIANT", "pl")
_DISPATCH = {"dp": _kernel_dp, "cc": _kernel_cc, "pl": _kernel_pl}


def kernel(x, W_ih, W_hh, b_ih, b_hh, W_fc, b_fc):
    x = np.asarray(x); W_ih = np.asarray(W_ih); W_hh = np.asarray(W_hh)
    b_ih = np.asarray(b_ih); b_hh = np.asarray(b_hh)
    W_fc = np.asarray(W_fc); b_fc = np.asarray(b_fc)
    return _DISPATCH[_VARIANT](x, W_ih, W_hh, b_ih, b_hh, W_fc, b_fc)

